# revision 5
# baseline (speedup 1.0000x reference)
"""Multi-head GAT layer on 8 Trainium2 NeuronCores (Bass/Tile SPMD kernel).

Strategy (edge-parallel, target-sharded):
  - Edges are sorted by target and sharded across 8 cores by contiguous
    target ranges (N/8 nodes per core), so each core owns all incoming
    edges of its targets: softmax + aggregation need no cross-core comms.
  - Phase 1 (replicated): every core computes the full projection table
    h = NF @ W.T + b  ->  h_table [N, H*F] in DRAM, plus per-node attention
    scores s12 = NF @ (W.T @ A12) + b12 -> s12_table [N, 2H].
  - Phase 2 (per core): per 128-target block, per 128-edge tile:
    gather h[src] rows (512B) + s12[tgt]/s12[src] rows via indirect DMA,
    compute ex = exp(leakyrelu(s1+s2)), Msg = ex*h_src, and accumulate
    both the weighted sum and the softmax denominator with one PE matmul
    per tile (lhsT = one-hot(edge->local target row), rhs = [Msg | ex]).
    Softmax division is pulled out of the edge loop: out = acc/denom +
    deg*h_own, then ELU.
"""

import numpy as np

N_CORES = 8
_last_results = None  # BassKernelResults of the most recent run (for harnesses)


def _install_ntff_hook():
    """Register the axon NTFF profiling hook if the image lacks antenv.axon_hooks."""
    import sys, types
    try:
        from antenv.axon_hooks import get_axon_ntff_profile_hook  # noqa: F401
        return
    except ImportError:
        pass
    try:
        mod = types.ModuleType("antenv.axon_hooks")
        holder = [None]
        mod.set_axon_ntff_profile_hook = lambda h: holder.__setitem__(0, h)
        mod.get_axon_ntff_profile_hook = lambda: holder[0]
        sys.modules["antenv.axon_hooks"] = mod
        from trn_agent_boot.trn_boot import _ntff_profile_via_ctypes
        mod.set_axon_ntff_profile_hook(
            _ntff_profile_via_ctypes("/opt/axon/libaxon_pjrt.so"))
    except Exception:
        sys.modules.pop("antenv.axon_hooks", None)


def kernel(node_features, edge_index, W, b, a):
    return gat_multicore(
        np.asarray(node_features, dtype=np.float32),
        np.asarray(edge_index, dtype=np.int32),
        np.asarray(W, dtype=np.float32),
        np.asarray(b, dtype=np.float32),
        np.asarray(a, dtype=np.float32),
    )


def gat_multicore(nf, ei, W, b, a, slope=0.2):
    import sys
    if "/opt/trn_rl_repo" not in sys.path:
        sys.path.insert(0, "/opt/trn_rl_repo")
    import concourse.bacc as bacc
    import concourse.tile as tile
    import concourse.mybir as mybir
    from concourse.bass import IndirectOffsetOnAxis
    from concourse.bass_utils import run_bass_kernel_spmd
    from contextlib import ExitStack

    fp32 = mybir.dt.float32
    i32 = mybir.dt.int32
    AF = mybir.ActivationFunctionType
    OP = mybir.AluOpType

    N, F_IN = nf.shape
    E = ei.shape[1]
    HF = W.shape[0]               # H * F_OUT
    F_OUT = a.shape[0] // 2
    H = HF // F_OUT
    assert F_IN == 128 and HF == 128, "kernel assumes 128 in/out features"
    assert N % N_CORES == 0
    NPC = N // N_CORES            # targets per core
    NBLK = (NPC + 127) // 128     # 128-target blocks per core
    GRP = 8                       # tiles per batching group

    # ---------------- host prep: weights ----------------
    WT = np.ascontiguousarray(W.T)                       # [F_IN, HF]
    A12 = np.zeros((HF, 2 * H), dtype=np.float32)
    for hd in range(H):
        A12[hd * F_OUT:(hd + 1) * F_OUT, hd] = a[:F_OUT]
        A12[hd * F_OUT:(hd + 1) * F_OUT, H + hd] = a[F_OUT:]
    M12 = (WT @ A12).astype(np.float32)                  # [F_IN, 2H]
    b12 = (b @ A12).astype(np.float32)                   # [2H]
    b_ext = np.concatenate([b, b12]).astype(np.float32)  # [HF + 2H]
    b_rep = np.broadcast_to(b_ext, (128, HF + 2 * H)).copy()
    NFT = np.ascontiguousarray(nf.T)                     # [F_IN, N]
    iota_rep = np.broadcast_to(
        np.arange(128, dtype=np.float32), (128, 128)).copy()

    # ---------------- host prep: graph structure ----------------
    src, tgt = ei[0].astype(np.int64), ei[1].astype(np.int64)
    order = np.argsort(tgt, kind="stable")
    ssrc, stgt = src[order], tgt[order]
    deg_full = np.bincount(tgt, minlength=N).astype(np.float32)

    # per (core, block) edge slices
    blk_bounds = []
    for c in range(N_CORES):
        bounds = [c * NPC + bb * 128 for bb in range(NBLK)] + [(c + 1) * NPC]
        blk_bounds.append(np.searchsorted(stgt, bounds))
    cnt = np.array([[blk_bounds[c][bb + 1] - blk_bounds[c][bb]
                     for bb in range(NBLK)] for c in range(N_CORES)])
    n_tiles_blk = np.maximum(1, (cnt.max(axis=0) + 127) // 128)  # uniform across cores
    NT = int(n_tiles_blk.sum())                                   # tiles per core

    # per-core slot arrays, [128, NT] layout: slot (p, t) = tile t, partition p
    srcs_np = np.zeros((N_CORES, 128, NT), dtype=np.int32)
    tgts_np = np.zeros((N_CORES, 128, NT), dtype=np.int32)
    rowid_np = np.full((N_CORES, 128, NT), -1.0, dtype=np.float32)
    degs_np = np.zeros((N_CORES, NBLK, 128), dtype=np.float32)
    ownid_np = np.zeros((N_CORES, NBLK, 128), dtype=np.int32)

    t_ofs_blk = np.concatenate([[0], np.cumsum(n_tiles_blk)]).astype(int)
    for c in range(N_CORES):
        for bb in range(NBLK):
            lo, hi = blk_bounds[c][bb], blk_bounds[c][bb + 1]
            nslot = hi - lo
            base_node = c * NPC + bb * 128
            nrows = min(128, (c + 1) * NPC - base_node)
            degs_np[c, bb, :nrows] = deg_full[base_node:base_node + nrows]
            ownid_np[c, bb, :nrows] = np.arange(base_node, base_node + nrows)
            if nslot > 0:
                o2 = np.argsort(ssrc[lo:hi], kind="stable")  # sort block by src
                s_blk = ssrc[lo:hi][o2]
                t_blk = stgt[lo:hi][o2]
                t0 = t_ofs_blk[bb]
                fl_s = np.zeros(n_tiles_blk[bb] * 128, dtype=np.int32)
                fl_t = np.zeros(n_tiles_blk[bb] * 128, dtype=np.int32)
                fl_r = np.full(n_tiles_blk[bb] * 128, -1.0, dtype=np.float32)
                fl_s[:nslot] = s_blk
                fl_t[:nslot] = t_blk
                fl_r[:nslot] = (t_blk - base_node).astype(np.float32)
                sl = slice(t0, t0 + n_tiles_blk[bb])
                srcs_np[c, :, sl] = fl_s.reshape(n_tiles_blk[bb], 128).T
                tgts_np[c, :, sl] = fl_t.reshape(n_tiles_blk[bb], 128).T
                rowid_np[c, :, sl] = fl_r.reshape(n_tiles_blk[bb], 128).T

    # ---------------- build the SPMD program ----------------
    nc = bacc.Bacc("TRN2", target_bir_lowering=False, debug=False,
                   num_devices=N_CORES)

    nft_d = nc.dram_tensor("nft", [128, N], fp32, kind="ExternalInput").ap()
    wt_d = nc.dram_tensor("wt", [128, HF], fp32, kind="ExternalInput").ap()
    m12_d = nc.dram_tensor("m12", [128, 2 * H], fp32, kind="ExternalInput").ap()
    brep_d = nc.dram_tensor("brep", [128, HF + 2 * H], fp32, kind="ExternalInput").ap()
    iota_d = nc.dram_tensor("iota", [128, 128], fp32, kind="ExternalInput").ap()
    srcs_d = nc.dram_tensor("srcs", [128, NT], i32, kind="ExternalInput").ap()
    tgts_d = nc.dram_tensor("tgts", [128, NT], i32, kind="ExternalInput").ap()
    rowid_d = nc.dram_tensor("rowid", [128, NT], fp32, kind="ExternalInput").ap()
    degs_d = nc.dram_tensor("degs", [NBLK, 128], fp32, kind="ExternalInput").ap()
    ownid_d = nc.dram_tensor("ownid", [NBLK, 128], i32, kind="ExternalInput").ap()

    h_tab = nc.dram_tensor("h_tab", [N, HF], fp32).ap()
    s12_tab = nc.dram_tensor("s12_tab", [N, 2 * H], fp32).ap()
    out_d = nc.dram_tensor("out", [NPC, HF], fp32, kind="ExternalOutput").ap()

    SW = HF + 2 * H   # 144: h row + s12 row in phase-1 psum
    CW = HF + H       # 136: Msg | ex combo width in phase-2

    with tile.TileContext(nc) as tc:
        with ExitStack() as ctx:
            cpool = ctx.enter_context(tc.tile_pool(name="consts", bufs=1))
            p1 = ctx.enter_context(tc.tile_pool(name="p1", bufs=3))
            p1ps = ctx.enter_context(tc.tile_pool(name="p1ps", bufs=2, space="PSUM"))
            gp = ctx.enter_context(tc.tile_pool(name="gather", bufs=3))
            mp = ctx.enter_context(tc.tile_pool(name="meta", bufs=3))
            ps2 = ctx.enter_context(tc.tile_pool(name="ps2", bufs=2, space="PSUM"))
            fin = ctx.enter_context(tc.tile_pool(name="fin", bufs=2))

            wt_sb = cpool.tile([128, HF], fp32)
            nc.sync.dma_start(wt_sb[:], wt_d[:])
            m12_sb = cpool.tile([128, 2 * H], fp32)
            nc.sync.dma_start(m12_sb[:], m12_d[:])
            brep_sb = cpool.tile([128, SW], fp32)
            nc.sync.dma_start(brep_sb[:], brep_d[:])
            iota_sb = cpool.tile([128, 128], fp32)
            nc.sync.dma_start(iota_sb[:], iota_d[:])

            # ---------- phase 1: h + s12 tables (replicated) ----------
            CH = 512
            for j0 in range(0, N, CH):
                w = min(CH, N - j0)
                nfc = p1.tile([128, CH], fp32, tag="nfc")
                nc.sync.dma_start(nfc[:, :w], nft_d[:, j0:j0 + w])
                for k0 in range(0, w, 128):
                    kw = min(128, w - k0)
                    ps = p1ps.tile([128, SW], fp32, space="PSUM", tag="p1ps")
                    nc.tensor.matmul(ps[:kw, 0:HF], lhsT=nfc[:, k0:k0 + kw],
                                     rhs=wt_sb[:], start=True, stop=True)
                    nc.tensor.matmul(ps[:kw, HF:SW], lhsT=nfc[:, k0:k0 + kw],
                                     rhs=m12_sb[:], start=True, stop=True)
                    hs = p1.tile([128, SW], fp32, tag="hs")
                    nc.vector.tensor_tensor(out=hs[:kw, :], in0=ps[:kw, :],
                                            in1=brep_sb[:kw, :], op=OP.add)
                    n0 = j0 + k0
                    nc.sync.dma_start(h_tab[n0:n0 + kw, :], hs[:kw, 0:HF])
                    nc.sync.dma_start(s12_tab[n0:n0 + kw, :], hs[:kw, HF:SW])

            # ---------- phase 2: edge processing ----------
            for bb in range(NBLK):
                ntb = int(n_tiles_blk[bb])
                t0 = int(t_ofs_blk[bb])
                base_row = bb * 128
                nrows = min(128, NPC - base_row)

                acc = ps2.tile([128, CW], fp32, space="PSUM", tag="acc")
                mm_i = 0
                for g0 in range(0, ntb, GRP):
                    gw = min(GRP, ntb - g0)
                    srct = mp.tile([128, GRP], i32, tag="srct")
                    nc.sync.dma_start(srct[:, :gw], srcs_d[:, t0 + g0:t0 + g0 + gw])
                    tgtt = mp.tile([128, GRP], i32, tag="tgtt")
                    nc.sync.dma_start(tgtt[:, :gw], tgts_d[:, t0 + g0:t0 + g0 + gw])
                    ridt = mp.tile([128, GRP], fp32, tag="ridt")
                    nc.sync.dma_start(ridt[:, :gw], rowid_d[:, t0 + g0:t0 + g0 + gw])

                    G = gp.tile([128, GRP, HF], fp32, tag="G")
                    sg = gp.tile([128, GRP, 4 * H], fp32, tag="sg")
                    oh = gp.tile([128, GRP, 128], fp32, tag="oh")
                    combo = gp.tile([128, GRP, CW], fp32, tag="combo")
                    for g in range(gw):
                        nc.gpsimd.indirect_dma_start(
                            out=G[:, g, :], out_offset=None, in_=h_tab[:, :],
                            in_offset=IndirectOffsetOnAxis(ap=srct[:, g:g + 1], axis=0))
                        nc.gpsimd.indirect_dma_start(
                            out=sg[:, g, 0:2 * H], out_offset=None, in_=s12_tab[:, :],
                            in_offset=IndirectOffsetOnAxis(ap=tgtt[:, g:g + 1], axis=0))
                        nc.gpsimd.indirect_dma_start(
                            out=sg[:, g, 2 * H:4 * H], out_offset=None, in_=s12_tab[:, :],
                            in_offset=IndirectOffsetOnAxis(ap=srct[:, g:g + 1], axis=0))

                    # one-hot of local target row (batched over the group)
                    nc.vector.tensor_tensor(
                        out=oh[:, :gw, :],
                        in0=ridt[:, :gw].unsqueeze(2).broadcast_to([128, gw, 128]),
                        in1=iota_sb[:].unsqueeze(1).broadcast_to([128, gw, 128]),
                        op=OP.is_equal)
                    # z = s1[tgt] + s2[src]  (strided slices of sg)
                    nc.vector.tensor_tensor(
                        out=combo[:, :gw, HF:CW],
                        in0=sg[:, :gw, 0:H], in1=sg[:, :gw, 3 * H:4 * H], op=OP.add)
                    # ex = exp(leakyrelu(z)); leakyrelu = max(slope*z, z) on DVE
                    nc.vector.scalar_tensor_tensor(
                        out=combo[:, :gw, HF:CW], in0=combo[:, :gw, HF:CW],
                        scalar=slope, in1=combo[:, :gw, HF:CW],
                        op0=OP.mult, op1=OP.max)
                    nc.scalar.activation(combo[:, :gw, HF:CW], combo[:, :gw, HF:CW],
                                         AF.Exp)
                    # Msg = ex (per head) * gathered h[src]
                    nc.vector.tensor_tensor(
                        out=combo[:, :gw, 0:HF],
                        in0=G[:, :gw, :],
                        in1=combo[:, :gw, HF:CW].unsqueeze(3).broadcast_to(
                            [128, gw, H, F_OUT]),
                        op=OP.mult)
                    for g in range(gw):
                        nc.tensor.matmul(acc[:, :], lhsT=oh[:, g, :],
                                         rhs=combo[:, g, :],
                                         start=(mm_i == 0), stop=(mm_i == ntb - 1))
                        mm_i += 1

                # ---------- finalize block ----------
                degt = fin.tile([128, 1], fp32, tag="degt")
                nc.sync.dma_start(degt[:nrows, :],
                                  degs_d[bb, :nrows].unsqueeze(1))
                ownt = fin.tile([128, 1], i32, tag="ownt")
                nc.sync.dma_start(ownt[:nrows, :],
                                  ownid_d[bb, :nrows].unsqueeze(1))
                h_own = fin.tile([128, HF], fp32, tag="h_own")
                nc.gpsimd.indirect_dma_start(
                    out=h_own[:, :], out_offset=None, in_=h_tab[:, :],
                    in_offset=IndirectOffsetOnAxis(ap=ownt[:, 0:1], axis=0))

                rec = fin.tile([128, H], fp32, tag="rec")
                nc.vector.tensor_scalar_add(out=rec[:, :], in0=acc[:, HF:CW],
                                            scalar1=1e-30)
                nc.vector.reciprocal(rec[:, :], rec[:, :])
                nrm = fin.tile([128, HF], fp32, tag="nrm")
                nc.vector.tensor_tensor(
                    out=nrm[:, :], in0=acc[:, 0:HF],
                    in1=rec[:].unsqueeze(2).broadcast_to([128, H, F_OUT]),
                    op=OP.mult)
                # += deg * h_own ; then ELU = max(x,0) + exp(min(x,0)) - 1
                nc.vector.scalar_tensor_tensor(
                    out=nrm[:, :], in0=h_own[:, :], scalar=degt[:, 0:1],
                    in1=nrm[:, :], op0=OP.mult, op1=OP.add)
                neg = fin.tile([128, HF], fp32, tag="neg")
                nc.vector.tensor_scalar_min(out=neg[:, :], in0=nrm[:, :], scalar1=0.0)
                nc.scalar.activation(neg[:, :], neg[:, :], AF.Exp)
                pos = fin.tile([128, HF], fp32, tag="pos")
                nc.vector.tensor_scalar_max(out=pos[:, :], in0=nrm[:, :], scalar1=0.0)
                res = fin.tile([128, HF], fp32, tag="res")
                nc.vector.scalar_tensor_tensor(
                    out=res[:, :], in0=neg[:, :], scalar=-1.0, in1=pos[:, :],
                    op0=OP.add, op1=OP.add)
                nc.sync.dma_start(out_d[base_row:base_row + nrows, :],
                                  res[:nrows, :])

    nc.compile()

    in_maps = []
    for c in range(N_CORES):
        in_maps.append({
            "nft": NFT, "wt": WT, "m12": M12, "brep": b_rep, "iota": iota_rep,
            "srcs": srcs_np[c], "tgts": tgts_np[c], "rowid": rowid_np[c],
            "degs": degs_np[c], "ownid": ownid_np[c],
        })
    import os
    trace = bool(os.environ.get("GAT_TRACE"))
    if trace:
        _install_ntff_hook()
    res = run_bass_kernel_spmd(nc, in_maps, list(range(N_CORES)), trace=trace)
    global _last_results
    _last_results = res
    out = np.concatenate([res.results[c]["out"] for c in range(N_CORES)], axis=0)
    return out


# revision 8
# speedup vs baseline: 3.3810x; 3.3810x over previous
"""Multi-head GAT layer on 8 Trainium2 NeuronCores (Bass/Tile SPMD kernel).

Strategy (edge-parallel, target-sharded):
  - Edges sorted by target, sharded across 8 cores by contiguous target
    ranges (N/8 nodes each): softmax + aggregation are core-local.
  - Phase 1 (replicated on every core): one bf16 PE pass over the node
    features builds an augmented per-node table row
      [ h (128) | s2 (8) | s1 (8) | deg (1) | pad ]  (bf16, 512B rows)
    where h = NF @ W.T + b and s1/s2 are the per-node attention scores
    h . a1 / h . a2 (fused into the same matmul via W.T @ A12).
  - Phase 2: per 128-target block, edge slots (padded to 128-slot tiles,
    sorted by src) are fetched with dma_gather (int16 indices + a static
    per-group base, 4 SWDGE queues round-robin).  Per tile, a one-hot
    matrix maps slots to local target rows; PE matmuls accumulate both
    the weighted message sum and the softmax denominator in PSUM.  The
    softmax division is pulled out of the edge loop (out = acc/denom);
    the skip term (deg * h_i) rides along as a per-target self-slot
    whose edge weight is deg * denom, so it survives the division
    exactly.  ELU finalize in fp32; contiguous output writes.
"""

import numpy as np

N_CORES = 8
_last_results = None  # BassKernelResults of the most recent run (for harnesses)


def _install_ntff_hook():
    """Register the axon NTFF profiling hook if the image lacks antenv.axon_hooks."""
    import sys, types
    try:
        from antenv.axon_hooks import get_axon_ntff_profile_hook  # noqa: F401
        return
    except ImportError:
        pass
    try:
        mod = types.ModuleType("antenv.axon_hooks")
        holder = [None]
        mod.set_axon_ntff_profile_hook = lambda h: holder.__setitem__(0, h)
        mod.get_axon_ntff_profile_hook = lambda: holder[0]
        sys.modules["antenv.axon_hooks"] = mod
        from trn_agent_boot.trn_boot import _ntff_profile_via_ctypes
        mod.set_axon_ntff_profile_hook(
            _ntff_profile_via_ctypes("/opt/axon/libaxon_pjrt.so"))
    except Exception:
        sys.modules.pop("antenv.axon_hooks", None)


def kernel(node_features, edge_index, W, b, a):
    return gat_multicore(
        np.asarray(node_features, dtype=np.float32),
        np.asarray(edge_index, dtype=np.int32),
        np.asarray(W, dtype=np.float32),
        np.asarray(b, dtype=np.float32),
        np.asarray(a, dtype=np.float32),
    )


def gat_multicore(nf, ei, W, b, a, slope=0.2):
    import sys
    if "/opt/trn_rl_repo" not in sys.path:
        sys.path.insert(0, "/opt/trn_rl_repo")
    import ml_dtypes
    import concourse.bacc as bacc
    import concourse.tile as tile
    import concourse.mybir as mybir
    from concourse import library_config
    from concourse.bass import IndirectOffsetOnAxis
    from concourse.bass_utils import run_bass_kernel_spmd
    from contextlib import ExitStack

    fp32 = mybir.dt.float32
    bf16 = mybir.dt.bfloat16
    i32 = mybir.dt.int32
    i16 = mybir.dt.int16
    AF = mybir.ActivationFunctionType
    OP = mybir.AluOpType
    bfnp = ml_dtypes.bfloat16

    N, F_IN = nf.shape
    E = ei.shape[1]
    HF = W.shape[0]               # H * F_OUT
    F_OUT = a.shape[0] // 2
    H = HF // F_OUT
    assert F_IN == 128 and HF == 128, "kernel assumes 128 in/out features"
    assert N % N_CORES == 0
    NPC = N // N_CORES            # targets per core
    NBLK = (NPC + 127) // 128     # 128-target blocks per core
    GRP = 8                       # max tiles per gather group
    ROW = 256                     # bf16 elements per table row (512 B)
    SPAN = 30000                  # max int16 index span per gather group

    # ---------------- host prep: weights ----------------
    WT = np.ascontiguousarray(W.T)                       # [F_IN, HF]
    # A12 column order: [s2 (a2) | s1 (a1)] to match the table row layout
    A12 = np.zeros((HF, 2 * H), dtype=np.float32)
    for hd in range(H):
        A12[hd * F_OUT:(hd + 1) * F_OUT, hd] = a[F_OUT:]        # s2
        A12[hd * F_OUT:(hd + 1) * F_OUT, H + hd] = a[:F_OUT]    # s1
    M12 = (WT @ A12).astype(np.float32)                  # [F_IN, 2H]
    b12 = (b @ A12).astype(np.float32)                   # [2H]
    b_ext = np.concatenate([b, b12]).astype(np.float32)  # [144]
    b_rep = np.broadcast_to(b_ext, (128, HF + 2 * H)).copy()
    NFT = np.ascontiguousarray(nf.T).astype(bfnp)        # [F_IN, N] bf16
    iota_rep = np.broadcast_to(
        np.arange(128, dtype=np.float32), (128, 128)).astype(bfnp).copy()
    ident = np.eye(128, dtype=np.float32).astype(bfnp)

    # ---------------- host prep: graph structure ----------------
    src, tgt = ei[0].astype(np.int64), ei[1].astype(np.int64)
    order = np.argsort(tgt, kind="stable")
    ssrc, stgt = src[order], tgt[order]
    deg_full = np.bincount(tgt, minlength=N).astype(np.float32)
    n_nt = (N + 127) // 128
    deg_pad = np.zeros(n_nt * 128, dtype=np.float32)
    deg_pad[:N] = deg_full
    deg_wrap = deg_pad.reshape(n_nt, 128).T.astype(bfnp).copy()

    blk_bounds = []
    for c in range(N_CORES):
        bounds = [c * NPC + bb * 128 for bb in range(NBLK)] + [(c + 1) * NPC]
        blk_bounds.append(np.searchsorted(stgt, bounds))
    cnt = np.array([[blk_bounds[c][bb + 1] - blk_bounds[c][bb]
                     for bb in range(NBLK)] for c in range(N_CORES)])
    # edge tiles per block (uniform across cores), +1 self tile
    n_edge_tiles = np.maximum(1, (cnt.max(axis=0) + 127) // 128)
    n_tiles_blk = n_edge_tiles + 1
    NT = int(n_tiles_blk.sum())
    t_ofs_blk = np.concatenate([[0], np.cumsum(n_tiles_blk)]).astype(int)

    # Per-core slot arrays; tile t slot p = slot index t*128+p of the block.
    # Last tile of each block is the self-slot tile (gathered by indirect DMA).
    srcs_all = np.zeros((N_CORES, 128, NT), dtype=np.int64)
    rowid_np = np.full((N_CORES, 128, NT), -1.0, dtype=np.float32)
    ownid_np = np.zeros((N_CORES, NBLK, 128), dtype=np.int32)
    for c in range(N_CORES):
        for bb in range(NBLK):
            lo, hi = blk_bounds[c][bb], blk_bounds[c][bb + 1]
            nslot = hi - lo
            base_node = c * NPC + bb * 128
            nrows = min(128, (c + 1) * NPC - base_node)
            t0 = int(t_ofs_blk[bb])
            net = int(n_edge_tiles[bb])
            ne = net * 128
            if nslot > 0:
                o2 = np.argsort(ssrc[lo:hi], kind="stable")
                s_blk = ssrc[lo:hi][o2]
                pad_val = int(s_blk[-1])
                fl_s = np.full(ne, pad_val, dtype=np.int64)
                fl_r = np.full(ne, -1.0, dtype=np.float32)
                fl_s[:nslot] = s_blk
                fl_r[:nslot] = (stgt[lo:hi][o2] - base_node).astype(np.float32)
                srcs_all[c, :, t0:t0 + net] = fl_s.reshape(net, 128).T
                rowid_np[c, :, t0:t0 + net] = fl_r.reshape(net, 128).T
            # else: pad_val filled below from other cores
            own = np.arange(128)
            valid = own < nrows
            ownid_np[c, bb] = np.where(valid, base_node + own, base_node)
            srcs_all[c, :, t0 + net] = 0  # unused (indirect gather path)
            rowid_np[c, :, t0 + net] = np.where(valid, own.astype(np.float32), -1.0)
    # blocks empty on some core but not others: align pad values to the
    # cross-core span by borrowing any non-empty core's pad value
    for bb in range(NBLK):
        t0 = int(t_ofs_blk[bb])
        net = int(n_edge_tiles[bb])
        nonempty = [c for c in range(N_CORES) if cnt[c][bb] > 0]
        if nonempty and len(nonempty) < N_CORES:
            ref = int(srcs_all[nonempty[0], 0, t0])
            for c in range(N_CORES):
                if cnt[c][bb] == 0:
                    srcs_all[c, :, t0:t0 + net] = ref

    # Gather groups over EDGE tiles only: consecutive tiles of one block,
    # <= GRP tiles, cross-core index span <= SPAN.  Base per group is the
    # cross-core min -> static program constant.
    groups = []          # (block, tile_lo, n_tiles, base)
    for bb in range(NBLK):
        net = int(n_edge_tiles[bb])
        t0 = int(t_ofs_blk[bb])
        t = 0
        while t < net:
            best = 1
            for w in range(2, min(GRP, net - t) + 1):
                sl = srcs_all[:, :, t0 + t:t0 + t + w]
                if sl.max() - sl.min() > SPAN:
                    break
                best = w
            sl = srcs_all[:, :, t0 + t:t0 + t + best]
            assert sl.max() - sl.min() <= 32000, "single tile span too large"
            groups.append((bb, t, best, int(sl.min())))
            t += best

    g_cols = [(g[2] * 128) // 16 for g in groups]
    g_col_ofs = np.concatenate([[0], np.cumsum(g_cols)]).astype(int)
    IDXC = int(g_col_ofs[-1])
    idx16_np = np.zeros((N_CORES, 128, IDXC), dtype=np.int16)
    for c in range(N_CORES):
        for gi, (bb, tl, w, base) in enumerate(groups):
            t0 = int(t_ofs_blk[bb]) + tl
            rel = (srcs_all[c, :, t0:t0 + w] - base).astype(np.int16)  # [128, w]
            flat = rel.T.reshape(-1)                 # slot order t*128+p
            wrapped = flat.reshape(-1, 16).T         # [16, w*128/16]
            idx16_np[c, :, g_col_ofs[gi]:g_col_ofs[gi + 1]] = np.tile(wrapped, (8, 1))

    rowid_bf = rowid_np.astype(bfnp)

    # ---------------- build the SPMD program ----------------
    nc = bacc.Bacc("TRN2", target_bir_lowering=False, debug=False,
                   num_devices=N_CORES, num_swdge_queues=4)

    nft_d = nc.dram_tensor("nft", [128, n_nt * 128], bf16, kind="ExternalInput").ap()
    wt_d = nc.dram_tensor("wt", [128, HF], bf16, kind="ExternalInput").ap()
    m12_d = nc.dram_tensor("m12", [128, 2 * H], bf16, kind="ExternalInput").ap()
    brep_d = nc.dram_tensor("brep", [128, HF + 2 * H], fp32, kind="ExternalInput").ap()
    iota_d = nc.dram_tensor("iota", [128, 128], bf16, kind="ExternalInput").ap()
    ident_d = nc.dram_tensor("ident", [128, 128], bf16, kind="ExternalInput").ap()
    degw_d = nc.dram_tensor("degw", [128, n_nt], bf16, kind="ExternalInput").ap()
    idx16_d = nc.dram_tensor("idx16", [128, IDXC], i16, kind="ExternalInput").ap()
    rowid_d = nc.dram_tensor("rowid", [128, NT], bf16, kind="ExternalInput").ap()
    ownid_d = nc.dram_tensor("ownid", [NBLK, 128], i32, kind="ExternalInput").ap()

    h_tab = nc.dram_tensor("h_tab", [N, ROW], bf16).ap()
    out_d = nc.dram_tensor("out", [NPC, HF], fp32, kind="ExternalOutput").ap()

    CW = HF + H       # 136: [Msg | ex] combo width
    SW = HF + 2 * H   # 144: phase-1 psum width
    MAXT = int(n_tiles_blk.max())

    with tile.TileContext(nc) as tc:
        with ExitStack() as ctx:
            cpool = ctx.enter_context(tc.tile_pool(name="consts", bufs=1))
            p1 = ctx.enter_context(tc.tile_pool(name="p1", bufs=3))
            p1ps = ctx.enter_context(tc.tile_pool(name="p1ps", bufs=2, space="PSUM"))
            gp = ctx.enter_context(tc.tile_pool(name="gather", bufs=2))
            mp = ctx.enter_context(tc.tile_pool(name="meta", bufs=3))
            ps_acc = ctx.enter_context(tc.tile_pool(name="ps_acc", bufs=2, space="PSUM"))
            ps_z = ctx.enter_context(tc.tile_pool(name="ps_z", bufs=2, space="PSUM"))
            ps_t = ctx.enter_context(tc.tile_pool(name="ps_t", bufs=1, space="PSUM"))
            fin = ctx.enter_context(tc.tile_pool(name="fin", bufs=2))

            nc.gpsimd.load_library(library_config.mlp)

            wt_sb = cpool.tile([128, HF], bf16)
            nc.sync.dma_start(wt_sb[:], wt_d[:])
            m12_sb = cpool.tile([128, 2 * H], bf16)
            nc.sync.dma_start(m12_sb[:], m12_d[:])
            brep_sb = cpool.tile([128, SW], fp32)
            nc.sync.dma_start(brep_sb[:], brep_d[:])
            iota_sb = cpool.tile([128, 128], bf16)
            nc.sync.dma_start(iota_sb[:], iota_d[:])
            ident_sb = cpool.tile([128, 128], bf16)
            nc.sync.dma_start(ident_sb[:], ident_d[:])
            idx_sb = cpool.tile([128, IDXC], i16)
            nc.sync.dma_start(idx_sb[:], idx16_d[:])

            # ---------- phase 1: augmented h table (replicated) ----------
            CH = 512
            for j0 in range(0, N, CH):
                w = min(CH, N - j0)
                nfc = p1.tile([128, CH], bf16, tag="nfc")
                nc.sync.dma_start(nfc[:, :w], nft_d[:, j0:j0 + w])
                ncols = (w + 127) // 128
                degc = p1.tile([128, CH // 128], bf16, tag="degc")
                nc.sync.dma_start(degc[:, :ncols],
                                  degw_d[:, j0 // 128:j0 // 128 + ncols])
                for k0 in range(0, w, 128):
                    kw = min(128, w - k0)
                    ps = p1ps.tile([128, SW], fp32, space="PSUM", tag="p1ps")
                    nc.tensor.matmul(ps[:kw, 0:HF], lhsT=nfc[:, k0:k0 + kw],
                                     rhs=wt_sb[:], start=True, stop=True)
                    nc.tensor.matmul(ps[:kw, HF:SW], lhsT=nfc[:, k0:k0 + kw],
                                     rhs=m12_sb[:], start=True, stop=True)
                    hrow = p1.tile([128, ROW], bf16, tag="hrow")
                    nc.vector.tensor_tensor(out=hrow[:kw, 0:SW], in0=ps[:kw, :],
                                            in1=brep_sb[:kw, :], op=OP.add)
                    nc.vector.tensor_copy(hrow[:kw, SW:SW + 1],
                                          degc[:kw, k0 // 128:k0 // 128 + 1])
                    n0 = j0 + k0
                    nc.sync.dma_start(h_tab[n0:n0 + kw, :], hrow[:kw, :])

            # ---------- phase 2: edge processing ----------
            qn = 0
            for bb in range(NBLK):
                ntb = int(n_tiles_blk[bb])
                net = ntb - 1
                t0 = int(t_ofs_blk[bb])
                base_row = bb * 128
                nrows = min(128, NPC - base_row)
                Tself = net

                acc = ps_acc.tile([128, CW], fp32, space="PSUM", tag="acc")
                G = gp.tile([128, MAXT, ROW], bf16, tag="G")
                combo = gp.tile([128, MAXT, CW], bf16, tag="combo")
                oh = gp.tile([128, MAXT, 128], bf16, tag="oh")
                ridt = mp.tile([128, MAXT], bf16, tag="ridt")
                nc.sync.dma_start(ridt[:, :ntb], rowid_d[:, t0:t0 + ntb])

                # self tile gather (absolute int32 ids, per-core data)
                ownt = mp.tile([128, 1], i32, tag="ownt")
                nc.sync.dma_start(ownt[:, :], ownid_d[bb, :].unsqueeze(1))
                nc.gpsimd.indirect_dma_start(
                    out=G[:, Tself, :], out_offset=None, in_=h_tab[:, :],
                    in_offset=IndirectOffsetOnAxis(ap=ownt[:, 0:1], axis=0))

                for gi, (gbb, tl, wdt, base) in enumerate(groups):
                    if gbb != bb:
                        continue
                    nc.gpsimd.dma_gather(
                        out_ap=G[:, tl:tl + wdt, :],
                        in_ap=h_tab[base:, :],
                        idxs_ap=idx_sb[:, g_col_ofs[gi]:g_col_ofs[gi + 1]],
                        num_idxs=wdt * 128, num_idxs_reg=wdt * 128,
                        elem_size=ROW, queue_num=qn % 4)
                    qn += 1

                # one-hot (bf16) for every tile of the block, batched
                nc.vector.tensor_tensor(
                    out=oh[:, :ntb, :],
                    in0=ridt[:, :ntb].unsqueeze(2).broadcast_to([128, ntb, 128]),
                    in1=iota_sb[:].unsqueeze(1).broadcast_to([128, ntb, 128]),
                    op=OP.is_equal)

                # s1 of the block's targets from the self tile's rows
                s1_blk = G[:, Tself, SW - H:SW]          # [128, H] bf16

                # per-tile: transpose(one-hot) -> s1e matmul; += s2 matmul
                zps = ps_z.tile([128, MAXT, H], fp32, space="PSUM", tag="zps")
                for q0 in range(0, ntb, GRP):
                    qw = min(GRP, ntb - q0)
                    ohT_ps = ps_t.tile([128, GRP, 128], bf16, space="PSUM",
                                       tag="ohT_ps")
                    for t in range(q0, q0 + qw):
                        nc.tensor.transpose(ohT_ps[:, t - q0, :], in_=oh[:, t, :],
                                            identity=ident_sb[:])
                    ohT_sb = gp.tile([128, GRP, 128], bf16, tag="ohT_sb")
                    nc.vector.tensor_copy(ohT_sb[:, :qw, :], ohT_ps[:, :qw, :])
                    for t in range(q0, q0 + qw):
                        nc.tensor.matmul(zps[:, t, :], lhsT=ohT_sb[:, t - q0, :],
                                         rhs=s1_blk, start=True, stop=False)
                        nc.tensor.matmul(zps[:, t, :], lhsT=ident_sb[:],
                                         rhs=G[:, t, HF:HF + H],
                                         start=False, stop=True)

                # ex = exp(leakyrelu(z)) for all tiles (garbage for the self
                # tile, overwritten below)
                z_sb = mp.tile([128, MAXT, H], fp32, tag="z_sb")
                nc.vector.tensor_copy(z_sb[:, :ntb, :], zps[:, :ntb, :])
                nc.vector.scalar_tensor_tensor(
                    out=combo[:, :ntb, HF:CW], in0=z_sb[:, :ntb, :], scalar=slope,
                    in1=z_sb[:, :ntb, :], op0=OP.mult, op1=OP.max)
                nc.scalar.activation(combo[:, :ntb, HF:CW], combo[:, :ntb, HF:CW],
                                     AF.Exp)
                # Msg = ex * h_src for the edge tiles
                nc.vector.tensor_tensor(
                    out=combo[:, 0:net, 0:HF], in0=G[:, 0:net, 0:HF],
                    in1=combo[:, 0:net, HF:CW].unsqueeze(3).broadcast_to(
                        [128, net, H, F_OUT]),
                    op=OP.mult)

                for t in range(net):
                    nc.tensor.matmul(acc[:, :], lhsT=oh[:, t, :], rhs=combo[:, t, :],
                                     start=(t == 0), stop=False)

                # self tile: ex_self = deg * denom(so far); no denom contribution
                deg_col = fin.tile([128, 1], fp32, tag="deg_col")
                nc.vector.tensor_copy(deg_col[:], G[:, Tself, SW:SW + 1])
                nc.vector.tensor_scalar_mul(
                    out=combo[:, Tself, HF:CW], in0=acc[:, HF:CW],
                    scalar1=deg_col[:, 0:1])
                nc.vector.tensor_tensor(
                    out=combo[:, Tself, 0:HF], in0=G[:, Tself, 0:HF],
                    in1=combo[:, Tself, HF:CW].unsqueeze(2).broadcast_to(
                        [128, H, F_OUT]),
                    op=OP.mult)
                nc.vector.memset(combo[:, Tself, HF:CW], 0.0)
                nc.tensor.matmul(acc[:, :], lhsT=oh[:, Tself, :],
                                 rhs=combo[:, Tself, :], start=False, stop=True)

                # ---------- finalize ----------
                rec = fin.tile([128, H], fp32, tag="rec")
                nc.vector.tensor_scalar_add(out=rec[:, :], in0=acc[:, HF:CW],
                                            scalar1=1e-30)
                nc.vector.reciprocal(rec[:, :], rec[:, :])
                nrm = fin.tile([128, HF], fp32, tag="nrm")
                nc.vector.tensor_tensor(
                    out=nrm[:, :], in0=acc[:, 0:HF],
                    in1=rec[:].unsqueeze(2).broadcast_to([128, H, F_OUT]),
                    op=OP.mult)
                # ELU = max(x,0) + exp(min(x,0)) - 1
                neg = fin.tile([128, HF], fp32, tag="neg")
                nc.vector.tensor_scalar_min(out=neg[:, :], in0=nrm[:, :], scalar1=0.0)
                nc.scalar.activation(neg[:, :], neg[:, :], AF.Exp)
                pos = fin.tile([128, HF], fp32, tag="pos")
                nc.vector.tensor_scalar_max(out=pos[:, :], in0=nrm[:, :], scalar1=0.0)
                res = fin.tile([128, HF], fp32, tag="res")
                nc.vector.scalar_tensor_tensor(
                    out=res[:, :], in0=neg[:, :], scalar=-1.0, in1=pos[:, :],
                    op0=OP.add, op1=OP.add)
                nc.sync.dma_start(out_d[base_row:base_row + nrows, :],
                                  res[:nrows, :])

    nc.compile()

    in_maps = []
    for c in range(N_CORES):
        in_maps.append({
            "nft": _pad_cols(NFT, n_nt * 128), "wt": WT.astype(bfnp),
            "m12": M12.astype(bfnp), "brep": b_rep, "iota": iota_rep,
            "ident": ident, "degw": deg_wrap,
            "idx16": idx16_np[c], "rowid": rowid_bf[c], "ownid": ownid_np[c],
        })
    import os
    trace = bool(os.environ.get("GAT_TRACE"))
    if trace:
        _install_ntff_hook()
    res = run_bass_kernel_spmd(nc, in_maps, list(range(N_CORES)), trace=trace)
    global _last_results
    _last_results = res
    out = np.concatenate([res.results[c]["out"] for c in range(N_CORES)], axis=0)
    return out


def _pad_cols(arr, cols):
    if arr.shape[1] == cols:
        return arr
    out = np.zeros((arr.shape[0], cols), dtype=arr.dtype)
    out[:, :arr.shape[1]] = arr
    return out


# revision 12
# speedup vs baseline: 3.7790x; 1.1177x over previous
"""Multi-head GAT layer on 8 Trainium2 NeuronCores (Bass/Tile SPMD kernel).

Strategy (edge-parallel, target-sharded):
  - Edges sorted by target, sharded across 8 cores by contiguous target
    ranges (N/8 nodes each): softmax + aggregation are core-local.
  - Phase 1 (replicated on every core): one bf16 PE pass over the node
    features builds an augmented per-node table row
      [ h (128) | s2 (8) | s1 (8) | deg (1) | pad ]  (bf16, 512B rows)
    where h = NF @ W.T + b and s1/s2 are the per-node attention scores
    h . a1 / h . a2 (fused into the same matmul via W.T @ A12).
  - Phase 2: per 128-target block, edge slots (padded to 128-slot tiles,
    sorted by src) are fetched with dma_gather (int16 indices + a static
    per-group base, 4 SWDGE queues round-robin).  Per tile, a one-hot
    matrix maps slots to local target rows; PE matmuls accumulate both
    the weighted message sum and the softmax denominator in PSUM.  The
    softmax division is pulled out of the edge loop (out = acc/denom);
    the skip term (deg * h_i) rides along as a per-target self-slot
    whose edge weight is deg * denom, so it survives the division
    exactly.  ELU finalize in fp32; contiguous output writes.
"""

import numpy as np

N_CORES = 8
_last_results = None  # BassKernelResults of the most recent run (for harnesses)


def _install_ntff_hook():
    """Register the axon NTFF profiling hook if the image lacks antenv.axon_hooks."""
    import sys, types
    try:
        from antenv.axon_hooks import get_axon_ntff_profile_hook  # noqa: F401
        return
    except ImportError:
        pass
    try:
        mod = types.ModuleType("antenv.axon_hooks")
        holder = [None]
        mod.set_axon_ntff_profile_hook = lambda h: holder.__setitem__(0, h)
        mod.get_axon_ntff_profile_hook = lambda: holder[0]
        sys.modules["antenv.axon_hooks"] = mod
        from trn_agent_boot.trn_boot import _ntff_profile_via_ctypes
        mod.set_axon_ntff_profile_hook(
            _ntff_profile_via_ctypes("/opt/axon/libaxon_pjrt.so"))
    except Exception:
        sys.modules.pop("antenv.axon_hooks", None)


def kernel(node_features, edge_index, W, b, a):
    return gat_multicore(
        np.asarray(node_features, dtype=np.float32),
        np.asarray(edge_index, dtype=np.int32),
        np.asarray(W, dtype=np.float32),
        np.asarray(b, dtype=np.float32),
        np.asarray(a, dtype=np.float32),
    )


def gat_multicore(nf, ei, W, b, a, slope=0.2):
    import sys
    if "/opt/trn_rl_repo" not in sys.path:
        sys.path.insert(0, "/opt/trn_rl_repo")
    import ml_dtypes
    import concourse.bacc as bacc
    import concourse.tile as tile
    import concourse.mybir as mybir
    from concourse import library_config
    from concourse.bass import IndirectOffsetOnAxis
    from concourse.bass_utils import run_bass_kernel_spmd
    from contextlib import ExitStack

    fp32 = mybir.dt.float32
    bf16 = mybir.dt.bfloat16
    i32 = mybir.dt.int32
    i16 = mybir.dt.int16
    AF = mybir.ActivationFunctionType
    OP = mybir.AluOpType
    bfnp = ml_dtypes.bfloat16

    N, F_IN = nf.shape
    E = ei.shape[1]
    HF = W.shape[0]               # H * F_OUT
    F_OUT = a.shape[0] // 2
    H = HF // F_OUT
    assert F_IN == 128 and HF == 128, "kernel assumes 128 in/out features"
    assert N % N_CORES == 0
    NPC = N // N_CORES            # targets per core
    NBLK = (NPC + 127) // 128     # 128-target blocks per core
    GRP = 8                       # max tiles per gather group
    ROW = 256                     # bf16 elements per table row (512 B)
    SPAN = 30000                  # max int16 index span per gather group

    # ---------------- host prep: weights ----------------
    WT = np.ascontiguousarray(W.T)                       # [F_IN, HF]
    # A12 column order: [s2 (a2) | s1 (a1)] to match the table row layout
    A12 = np.zeros((HF, 2 * H), dtype=np.float32)
    for hd in range(H):
        A12[hd * F_OUT:(hd + 1) * F_OUT, hd] = a[F_OUT:]        # s2
        A12[hd * F_OUT:(hd + 1) * F_OUT, H + hd] = a[:F_OUT]    # s1
    M12 = (WT @ A12).astype(np.float32)                  # [F_IN, 2H]
    b12 = (b @ A12).astype(np.float32)                   # [2H]
    b_ext = np.concatenate([b, b12]).astype(np.float32)  # [144]
    b_rep = np.broadcast_to(b_ext, (128, HF + 2 * H)).copy()
    NFT = np.ascontiguousarray(nf.T).astype(bfnp)        # [F_IN, N] bf16
    iota_rep = np.broadcast_to(
        np.arange(128, dtype=np.float32), (128, 128)).astype(bfnp).copy()
    ident = np.eye(128, dtype=np.float32).astype(bfnp)

    # ---------------- host prep: graph structure ----------------
    src, tgt = ei[0].astype(np.int64), ei[1].astype(np.int64)
    order = np.argsort(tgt, kind="stable")
    ssrc, stgt = src[order], tgt[order]
    deg_full = np.bincount(tgt, minlength=N).astype(np.float32)
    n_nt = (N + 127) // 128
    deg_pad = np.zeros(n_nt * 128, dtype=np.float32)
    deg_pad[:N] = deg_full
    deg_wrap = deg_pad.reshape(n_nt, 128).T.astype(bfnp).copy()

    blk_bounds = []
    for c in range(N_CORES):
        bounds = [c * NPC + bb * 128 for bb in range(NBLK)] + [(c + 1) * NPC]
        blk_bounds.append(np.searchsorted(stgt, bounds))
    cnt = np.array([[blk_bounds[c][bb + 1] - blk_bounds[c][bb]
                     for bb in range(NBLK)] for c in range(N_CORES)])
    # edge tiles per block (uniform across cores), +1 self tile
    n_edge_tiles = np.maximum(1, (cnt.max(axis=0) + 127) // 128)
    n_tiles_blk = n_edge_tiles + 1
    NT = int(n_tiles_blk.sum())
    t_ofs_blk = np.concatenate([[0], np.cumsum(n_tiles_blk)]).astype(int)

    # Per-core slot arrays; tile t slot p = slot index t*128+p of the block.
    # Last tile of each block is the self-slot tile (gathered by indirect DMA).
    srcs_all = np.zeros((N_CORES, 128, NT), dtype=np.int64)
    rowid_np = np.full((N_CORES, 128, NT), -1.0, dtype=np.float32)
    ownid_np = np.zeros((N_CORES, NBLK, 128), dtype=np.int32)
    for c in range(N_CORES):
        for bb in range(NBLK):
            lo, hi = blk_bounds[c][bb], blk_bounds[c][bb + 1]
            nslot = hi - lo
            base_node = c * NPC + bb * 128
            nrows = min(128, (c + 1) * NPC - base_node)
            t0 = int(t_ofs_blk[bb])
            net = int(n_edge_tiles[bb])
            ne = net * 128
            if nslot > 0:
                o2 = np.argsort(ssrc[lo:hi], kind="stable")
                s_blk = ssrc[lo:hi][o2]
                pad_val = int(s_blk[-1])
                fl_s = np.full(ne, pad_val, dtype=np.int64)
                fl_r = np.full(ne, -1.0, dtype=np.float32)
                fl_s[:nslot] = s_blk
                fl_r[:nslot] = (stgt[lo:hi][o2] - base_node).astype(np.float32)
                srcs_all[c, :, t0:t0 + net] = fl_s.reshape(net, 128).T
                rowid_np[c, :, t0:t0 + net] = fl_r.reshape(net, 128).T
            # else: pad_val filled below from other cores
            own = np.arange(128)
            valid = own < nrows
            ownid_np[c, bb] = np.where(valid, base_node + own, base_node)
            srcs_all[c, :, t0 + net] = 0  # unused (indirect gather path)
            rowid_np[c, :, t0 + net] = np.where(valid, own.astype(np.float32), -1.0)
    # blocks empty on some core but not others: align pad values to the
    # cross-core span by borrowing any non-empty core's pad value
    for bb in range(NBLK):
        t0 = int(t_ofs_blk[bb])
        net = int(n_edge_tiles[bb])
        nonempty = [c for c in range(N_CORES) if cnt[c][bb] > 0]
        if nonempty and len(nonempty) < N_CORES:
            ref = int(srcs_all[nonempty[0], 0, t0])
            for c in range(N_CORES):
                if cnt[c][bb] == 0:
                    srcs_all[c, :, t0:t0 + net] = ref

    # Gather groups over EDGE tiles only: consecutive tiles of one block,
    # <= GRP tiles, cross-core index span <= SPAN.  Base per group is the
    # cross-core min -> static program constant.
    groups = []          # (block, tile_lo, n_tiles, base)
    for bb in range(NBLK):
        net = int(n_edge_tiles[bb])
        t0 = int(t_ofs_blk[bb])
        t = 0
        while t < net:
            best = 1
            for w in range(2, min(GRP, net - t) + 1):
                sl = srcs_all[:, :, t0 + t:t0 + t + w]
                if sl.max() - sl.min() > SPAN:
                    break
                best = w
            sl = srcs_all[:, :, t0 + t:t0 + t + best]
            assert sl.max() - sl.min() <= 32000, "single tile span too large"
            groups.append((bb, t, best, int(sl.min())))
            t += best

    g_cols = [(g[2] * 128) // 16 for g in groups]
    g_col_ofs = np.concatenate([[0], np.cumsum(g_cols)]).astype(int)
    IDXC = int(g_col_ofs[-1])
    idx16_np = np.zeros((N_CORES, 128, IDXC), dtype=np.int16)
    for c in range(N_CORES):
        for gi, (bb, tl, w, base) in enumerate(groups):
            t0 = int(t_ofs_blk[bb]) + tl
            rel = (srcs_all[c, :, t0:t0 + w] - base).astype(np.int16)  # [128, w]
            flat = rel.T.reshape(-1)                 # slot order t*128+p
            wrapped = flat.reshape(-1, 16).T         # [16, w*128/16]
            idx16_np[c, :, g_col_ofs[gi]:g_col_ofs[gi + 1]] = np.tile(wrapped, (8, 1))

    rowid_bf = rowid_np.astype(bfnp)

    # ---------------- build the SPMD program ----------------
    nc = bacc.Bacc("TRN2", target_bir_lowering=False, debug=False,
                   num_devices=N_CORES, num_swdge_queues=4)

    nft_d = nc.dram_tensor("nft", [128, n_nt * 128], bf16, kind="ExternalInput").ap()
    wt_d = nc.dram_tensor("wt", [128, HF], bf16, kind="ExternalInput").ap()
    m12_d = nc.dram_tensor("m12", [128, 2 * H], bf16, kind="ExternalInput").ap()
    brep_d = nc.dram_tensor("brep", [128, HF + 2 * H], fp32, kind="ExternalInput").ap()
    iota_d = nc.dram_tensor("iota", [128, 128], bf16, kind="ExternalInput").ap()
    ident_d = nc.dram_tensor("ident", [128, 128], bf16, kind="ExternalInput").ap()
    degw_d = nc.dram_tensor("degw", [128, n_nt], bf16, kind="ExternalInput").ap()
    idx16_d = nc.dram_tensor("idx16", [128, IDXC], i16, kind="ExternalInput").ap()
    rowid_d = nc.dram_tensor("rowid", [128, NT], bf16, kind="ExternalInput").ap()
    ownid_d = nc.dram_tensor("ownid", [NBLK, 128], i32, kind="ExternalInput").ap()

    h_tab = nc.dram_tensor("h_tab", [N, ROW], bf16).ap()
    out_d = nc.dram_tensor("out", [NPC, HF], fp32, kind="ExternalOutput").ap()

    CW = HF + H       # 136: [Msg | ex] combo width
    SW = HF + 2 * H   # 144: phase-1 psum width
    MAXT = int(n_tiles_blk.max())

    with tile.TileContext(nc) as tc:
        with ExitStack() as ctx:
            cpool = ctx.enter_context(tc.tile_pool(name="consts", bufs=1))
            p1 = ctx.enter_context(tc.tile_pool(name="p1", bufs=3))
            p1ps = ctx.enter_context(tc.tile_pool(name="p1ps", bufs=2, space="PSUM"))
            gp = ctx.enter_context(tc.tile_pool(name="gather", bufs=2))
            mp = ctx.enter_context(tc.tile_pool(name="meta", bufs=3))
            ps_acc = ctx.enter_context(tc.tile_pool(name="ps_acc", bufs=2, space="PSUM"))
            ps_z = ctx.enter_context(tc.tile_pool(name="ps_z", bufs=2, space="PSUM"))
            ps_t = ctx.enter_context(tc.tile_pool(name="ps_t", bufs=2, space="PSUM"))
            fin = ctx.enter_context(tc.tile_pool(name="fin", bufs=2))

            nc.gpsimd.load_library(library_config.mlp)

            wt_sb = cpool.tile([128, HF], bf16)
            nc.sync.dma_start(wt_sb[:], wt_d[:])
            m12_sb = cpool.tile([128, 2 * H], bf16)
            nc.sync.dma_start(m12_sb[:], m12_d[:])
            brep_sb = cpool.tile([128, SW], fp32)
            nc.sync.dma_start(brep_sb[:], brep_d[:])
            iota_sb = cpool.tile([128, 128], bf16)
            nc.sync.dma_start(iota_sb[:], iota_d[:])
            ident_sb = cpool.tile([128, 128], bf16)
            nc.sync.dma_start(ident_sb[:], ident_d[:])
            idx_sb = cpool.tile([128, IDXC], i16)
            nc.sync.dma_start(idx_sb[:], idx16_d[:])

            # ---------- phase 1: augmented h table (replicated) ----------
            b_is_zero = not np.any(b_ext)
            CH = 512
            for j0 in range(0, N, CH):
                w = min(CH, N - j0)
                nfc = p1.tile([128, CH], bf16, tag="nfc")
                nc.sync.dma_start(nfc[:, :w], nft_d[:, j0:j0 + w])
                ncols = (w + 127) // 128
                degc = p1.tile([128, CH // 128], bf16, tag="degc")
                nc.sync.dma_start(degc[:, :ncols],
                                  degw_d[:, j0 // 128:j0 // 128 + ncols])
                for k0 in range(0, w, 256):
                    # two node-tiles per hrow buffer / table write
                    kw2 = min(256, w - k0)
                    hrow = p1.tile([128, 2, ROW], bf16, tag="hrow")
                    nk = (kw2 + 127) // 128
                    ps = p1ps.tile([128, 2, SW], fp32, space="PSUM", tag="p1ps")
                    for k in range(nk):
                        kk = k0 + k * 128
                        kw = min(128, w - kk)
                        nc.tensor.matmul(ps[:kw, k, 0:HF],
                                         lhsT=nfc[:, kk:kk + kw],
                                         rhs=wt_sb[:], start=True, stop=True)
                        nc.tensor.matmul(ps[:kw, k, HF:SW],
                                         lhsT=nfc[:, kk:kk + kw],
                                         rhs=m12_sb[:], start=True, stop=True)
                    if b_is_zero:
                        nc.scalar.copy(hrow[:, :nk, 0:SW], ps[:, :nk, :])
                    else:
                        nc.vector.tensor_tensor(
                            out=hrow[:, :nk, 0:SW], in0=ps[:, :nk, :],
                            in1=brep_sb[:].unsqueeze(1).broadcast_to([128, nk, SW]),
                            op=OP.add)
                    nc.vector.tensor_copy(
                        hrow[:, :nk, SW:SW + 1],
                        degc[:, k0 // 128:k0 // 128 + nk].unsqueeze(2))
                    n0 = j0 + k0
                    if kw2 == nk * 128:
                        nc.scalar.dma_start(
                            h_tab[n0:n0 + kw2, :].rearrange(
                                "(k p) r -> p k r", k=nk),
                            hrow[:, :nk, :])
                    else:
                        nc.scalar.dma_start(h_tab[n0:n0 + kw2, :],
                                            hrow[:kw2, 0, :])

            # ---------- phase 2: edge processing ----------
            # Per block: main = gathers + scores + weighted-agg matmuls into
            # PSUM; tail = softmax division + skip + ELU + output write.
            # Tails are emitted one block late so their PSUM waits overlap
            # with the next block's compute.
            blk_state = {}

            def emit_main(bb, qn0):
                ntb = int(n_tiles_blk[bb])
                net = ntb - 1
                t0 = int(t_ofs_blk[bb])
                Tself = net
                qn = qn0

                acc = ps_acc.tile([128, CW], fp32, space="PSUM", tag="acc")
                G = gp.tile([128, MAXT, ROW], bf16, tag="G")
                combo = gp.tile([128, MAXT, CW], bf16, tag="combo")
                oh = gp.tile([128, MAXT, 128], bf16, tag="oh")
                ridt = mp.tile([128, MAXT], bf16, tag="ridt")
                nc.sync.dma_start(ridt[:, :ntb], rowid_d[:, t0:t0 + ntb])

                # self tile gather (absolute int32 ids, per-core data)
                ownt = mp.tile([128, 1], i32, tag="ownt")
                nc.sync.dma_start(ownt[:, :], ownid_d[bb, :].unsqueeze(1))
                nc.gpsimd.indirect_dma_start(
                    out=G[:, Tself, :], out_offset=None, in_=h_tab[:, :],
                    in_offset=IndirectOffsetOnAxis(ap=ownt[:, 0:1], axis=0))

                for gi, (gbb, tl, wdt, base) in enumerate(groups):
                    if gbb != bb:
                        continue
                    nc.gpsimd.dma_gather(
                        out_ap=G[:, tl:tl + wdt, :],
                        in_ap=h_tab[base:, :],
                        idxs_ap=idx_sb[:, g_col_ofs[gi]:g_col_ofs[gi + 1]],
                        num_idxs=wdt * 128, num_idxs_reg=wdt * 128,
                        elem_size=ROW, queue_num=qn % 4)
                    qn += 1

                # one-hot (bf16) for the edge tiles, batched
                nc.vector.tensor_tensor(
                    out=oh[:, :net, :],
                    in0=ridt[:, :net].unsqueeze(2).broadcast_to([128, net, 128]),
                    in1=iota_sb[:].unsqueeze(1).broadcast_to([128, net, 128]),
                    op=OP.is_equal)

                # s1 of the block's targets from the self tile's rows
                s1_blk = G[:, Tself, SW - H:SW]          # [128, H] bf16

                # per-tile: transpose(one-hot) -> s1e matmul
                zps = ps_z.tile([128, MAXT, H], fp32, space="PSUM", tag="zps")
                for q0 in range(0, net, GRP):
                    qw = min(GRP, net - q0)
                    ohT_ps = ps_t.tile([128, GRP, 128], bf16, space="PSUM",
                                       tag="ohT_ps")
                    for t in range(q0, q0 + qw):
                        nc.tensor.transpose(ohT_ps[:, t - q0, :], in_=oh[:, t, :],
                                            identity=ident_sb[:])
                    ohT_sb = gp.tile([128, GRP, 128], bf16, tag="ohT_sb")
                    nc.scalar.copy(ohT_sb[:, :qw, :], ohT_ps[:, :qw, :])
                    for t in range(q0, q0 + qw):
                        nc.tensor.matmul(zps[:, t, :], lhsT=ohT_sb[:, t - q0, :],
                                         rhs=s1_blk, start=True, stop=True)

                # ex = exp(leakyrelu(s1e + s2)); z = zps + G.s2 on DVE
                z_sb = mp.tile([128, MAXT, H], fp32, tag="z_sb")
                nc.vector.tensor_tensor(out=z_sb[:, :net, :], in0=zps[:, :net, :],
                                        in1=G[:, :net, HF:HF + H], op=OP.add)
                nc.vector.scalar_tensor_tensor(
                    out=combo[:, :net, HF:CW], in0=z_sb[:, :net, :], scalar=slope,
                    in1=z_sb[:, :net, :], op0=OP.mult, op1=OP.max)
                nc.scalar.activation(combo[:, :net, HF:CW], combo[:, :net, HF:CW],
                                     AF.Exp)
                # Msg = ex * h_src for the edge tiles
                nc.vector.tensor_tensor(
                    out=combo[:, 0:net, 0:HF], in0=G[:, 0:net, 0:HF],
                    in1=combo[:, 0:net, HF:CW].unsqueeze(3).broadcast_to(
                        [128, net, H, F_OUT]),
                    op=OP.mult)

                for t in range(net):
                    nc.tensor.matmul(acc[:, :], lhsT=oh[:, t, :], rhs=combo[:, t, :],
                                     start=(t == 0), stop=(t == net - 1))

                blk_state[bb] = (acc, G, Tself)
                return qn

            def emit_tail(bb):
                ntb = int(n_tiles_blk[bb])
                base_row = bb * 128
                nrows = min(128, NPC - base_row)
                acc, G, Tself = blk_state.pop(bb)

                rec = fin.tile([128, H], fp32, tag="rec")
                nc.vector.tensor_scalar_add(out=rec[:, :], in0=acc[:, HF:CW],
                                            scalar1=1e-30)
                nc.vector.reciprocal(rec[:, :], rec[:, :])
                nrm = fin.tile([128, HF], fp32, tag="nrm")
                nc.vector.tensor_tensor(
                    out=nrm[:, :], in0=acc[:, 0:HF],
                    in1=rec[:].unsqueeze(2).broadcast_to([128, H, F_OUT]),
                    op=OP.mult)
                # += deg * h_own (fp32, exact skip term)
                deg_col = fin.tile([128, 1], fp32, tag="deg_col")
                nc.vector.tensor_copy(deg_col[:], G[:, Tself, SW:SW + 1])
                nc.vector.scalar_tensor_tensor(
                    out=nrm[:, :], in0=G[:, Tself, 0:HF], scalar=deg_col[:, 0:1],
                    in1=nrm[:, :], op0=OP.mult, op1=OP.add)
                # ELU = max(x,0) + exp(min(x,0)) - 1
                neg = fin.tile([128, HF], fp32, tag="neg")
                nc.vector.tensor_scalar_min(out=neg[:, :], in0=nrm[:, :], scalar1=0.0)
                nc.scalar.activation(neg[:, :], neg[:, :], AF.Exp)
                pos = fin.tile([128, HF], fp32, tag="pos")
                nc.vector.tensor_scalar_max(out=pos[:, :], in0=nrm[:, :], scalar1=0.0)
                res = fin.tile([128, HF], fp32, tag="res")
                nc.vector.scalar_tensor_tensor(
                    out=res[:, :], in0=neg[:, :], scalar=-1.0, in1=pos[:, :],
                    op0=OP.add, op1=OP.add)
                nc.scalar.dma_start(out_d[base_row:base_row + nrows, :],
                                    res[:nrows, :])

            qn = 0
            for bb in range(NBLK):
                qn = emit_main(bb, qn)
                if bb > 0:
                    emit_tail(bb - 1)
            emit_tail(NBLK - 1)

    nc.compile()

    in_maps = []
    for c in range(N_CORES):
        in_maps.append({
            "nft": _pad_cols(NFT, n_nt * 128), "wt": WT.astype(bfnp),
            "m12": M12.astype(bfnp), "brep": b_rep, "iota": iota_rep,
            "ident": ident, "degw": deg_wrap,
            "idx16": idx16_np[c], "rowid": rowid_bf[c], "ownid": ownid_np[c],
        })
    import os
    trace = bool(os.environ.get("GAT_TRACE"))
    if trace:
        _install_ntff_hook()
    res = run_bass_kernel_spmd(nc, in_maps, list(range(N_CORES)), trace=trace)
    global _last_results
    _last_results = res
    out = np.concatenate([res.results[c]["out"] for c in range(N_CORES)], axis=0)
    return out


def _pad_cols(arr, cols):
    if arr.shape[1] == cols:
        return arr
    out = np.zeros((arr.shape[0], cols), dtype=arr.dtype)
    out[:, :arr.shape[1]] = arr
    return out
